# revision 17
# baseline (speedup 1.0000x reference)
"""Trainium2 Bass kernel for a 4-layer BiLSTM + FC + softmax.

Model (matches the PyTorch-style reference):
  4 stacked bidirectional LSTM layers, hidden sizes [50, 25, 25, 10],
  input sizes [3, 100, 50, 50], T=2048, B=256, then a (20 -> 2) linear
  layer and a softmax over the 2 classes, output reshaped to (T*B, 2).

Sharding: data-parallel over the batch axis. Each of the 8 NeuronCores
processes a batch slice of 32 elements; parameters are replicated.

Per-core layout is feature-major: every activation tensor is stored as
(features, T*32) with column index t*32 + b.  The LSTM recurrence for
direction d of layer l is computed with a single fused matmul per step:

    gates_t = lhsT.T @ [h_{t-1}; 1; x_t]     lhsT = [Whh.T; bias; Wih.T]

Engine access patterns must start at 32-aligned partitions, so the four
gate blocks are placed at quad offsets within the psum/sigmoid tiles
(padding columns in lhsT are zero; the pad costs nothing since op time
scales with the free dim).  All four gates go through one sigmoid with a
per-partition scale vector (g rows scaled by 2) so tanh(g) = 2*sig(2g)-1
is reconstructed on the vector engine; the scalar engine runs exactly 2
instructions per step per direction (sigmoid of gates, tanh of cell).

The h state is written (shifted one time-block) into the head of the
streaming input window, so one SBUF buffer serves as matmul input for
the next step and as the DMA source for the layer output writeback.

The final FC + softmax uses p0 = sigmoid(d+bd), p1 = sigmoid(-d-bd) with
d = (w0-w1).h, evaluated as tiny matmuls that put the flattened (t,b)
index on the partition axis so the sigmoids run 128-lane parallel.
"""

import os
import time

import numpy as np
from contextlib import ExitStack

import concourse.bass as bass
import concourse.bacc as bacc
import concourse.tile as tile
import concourse.mybir as mybir
from concourse.bass_utils import run_bass_kernel_spmd

F32 = mybir.dt.float32
F32R = mybir.dt.float32r
SIG = mybir.ActivationFunctionType.Sigmoid
TANH = mybir.ActivationFunctionType.Tanh
MULT = mybir.AluOpType.mult
SUB = mybir.AluOpType.subtract

HID = [50, 25, 25, 10]
INS = [3, 100, 50, 50]
NCORES = 8
B = 32  # batch per core

_prog_cache = {}


def _gate_layout(li):
    """Per psum tile: (height, [(gate_name, row_offset), ...]).

    Gate blocks sit at 32-aligned partition offsets. Layer 0 (H=50) splits
    into two tiles: [i@0, f@64] and [g@0, o@64]; others use one tile with
    i@0, f@32, g@64, o@96.
    """
    H = HID[li]
    if li == 0:
        return [(114, [("i", 0), ("f", 64)]), (114, [("g", 0), ("o", 64)])]
    return [(96 + H, [("i", 0), ("f", 32), ("g", 64), ("o", 96)])]


def build_program(T, C, reps=1):
    """Build the full SPMD Bass program for sequence length T, chunk C."""
    TB = T * B
    NCH = T // C
    assert T % C == 0

    nc = bacc.Bacc("TRN2", debug=False, target_bir_lowering=False,
                   num_devices=NCORES)

    # ---- DRAM tensors -------------------------------------------------
    x0 = nc.dram_tensor("x0", [3, TB], F32, kind="ExternalInput")
    w_in = {}
    b_in = {}
    for li in range(4):
        I, H = INS[li], HID[li]
        K = H + I
        for d in range(2):
            for hi, (ph, _) in enumerate(_gate_layout(li)):
                w_in[(li, d, hi)] = nc.dram_tensor(
                    f"w{li}_{d}_{hi}", [K, ph], F32, kind="ExternalInput")
                b_in[(li, d, hi)] = nc.dram_tensor(
                    f"b{li}_{d}_{hi}", [ph, 1], F32, kind="ExternalInput")
    sc_in = {}
    for li in range(4):
        ph0 = _gate_layout(li)[-1][0]
        sc_in[li] = nc.dram_tensor(f"sc{li}", [ph0, 1], F32,
                                   kind="ExternalInput")
    fcw = nc.dram_tensor("fcw", [20, 2], F32, kind="ExternalInput")
    fcb = nc.dram_tensor("fcb", [128, 2], F32, kind="ExternalInput")

    out = nc.dram_tensor("out", [2, TB], F32, kind="ExternalOutput")
    # inter-layer activations: rows [0:H] fwd h, rows [H:2H] bwd h
    XO = [nc.dram_tensor(f"xo{li}", [2 * HID[li], TB], F32)
          for li in range(4)]

    with tile.TileContext(nc, num_cores=NCORES) as tc, ExitStack() as top:
        if reps > 1:
            top.enter_context(tc.For_i(0, reps, 1))
        wpool = top.enter_context(tc.tile_pool(name="weights", bufs=1))
        wt = {}
        for key, dram in w_in.items():
            t = wpool.tile(list(dram.shape), F32, tag=f"w{key}",
                           name=f"wt_{key[0]}_{key[1]}_{key[2]}")
            nc.sync.dma_start(t[:], dram.ap())
            wt[key] = t
        bt = {}
        for key, dram in b_in.items():
            t = wpool.tile(list(dram.shape), F32, tag=f"b{key}",
                           name=f"bt_{key[0]}_{key[1]}_{key[2]}")
            nc.sync.dma_start(t[:], dram.ap())
            bt[key] = t
        sct = {}
        for li, dram in sc_in.items():
            t = wpool.tile(list(dram.shape), F32, tag=f"sc{li}",
                           name=f"sct_{li}")
            nc.sync.dma_start(t[:], dram.ap())
            sct[li] = t
        fcw_t = wpool.tile([20, 2], F32, tag="fcw")
        nc.sync.dma_start(fcw_t[:], fcw.ap())
        fcb_t = wpool.tile([128, 2], F32, tag="fcb")
        nc.sync.dma_start(fcb_t[:], fcb.ap())

        for li in range(4):
            _build_layer(nc, tc, li, T, C, NCH,
                         x0 if li == 0 else XO[li - 1], XO[li], wt, bt, sct)

        _build_fc(nc, tc, TB, XO[3], fcw_t, fcb_t, out)

    nc.compile()
    return nc


def _build_layer(nc, tc, li, T, C, NCH, xin, xo, wt, bt, sct):
    I, H = INS[li], HID[li]
    K = H + I
    CB = C * B
    layout = _gate_layout(li)

    with ExitStack() as ls:
        winp = ls.enter_context(tc.tile_pool(name=f"win{li}", bufs=1))
        spool = ls.enter_context(tc.tile_pool(name=f"sg{li}", bufs=3))
        tpool = ls.enter_context(tc.tile_pool(name=f"tm{li}", bufs=3))
        cpool = ls.enter_context(tc.tile_pool(name=f"cc{li}", bufs=1))
        pspools = {}
        for d in range(2):
            for hi in range(len(layout)):
                pspools[(d, hi)] = ls.enter_context(tc.tile_pool(
                    name=f"ps{li}_{d}_{hi}", bufs=2,
                    space=bass.MemorySpace.PSUM))

        # windows: [dir][buf] -> (K, C*B); rows [h(H); x(I)].
        # h written by compute (shifted one block); x DMA'd from DRAM.
        win = [[winp.tile([K, CB], F32, tag=f"w{d}{bf}",
                          name=f"win{li}_{d}_{bf}") for bf in range(2)]
               for d in range(2)]
        # initial h = 0
        nc.vector.memset(win[0][0][0:H, 0:B], 0.0)
        nc.vector.memset(win[1][0][0:H, (C - 1) * B:CB], 0.0)

        # Partition placement: DVE tensor_tensor requires equal base
        # partitions when both inputs are SBUF.  Gates sit at quad offsets
        # (L0: i@0,f@64 | g@0,o@64; others: i@0,f@32,g@64,o@96).  g~ is
        # rebased to 0 by the single-input tensor_scalar; u/m/c live at
        # f's offset; tau lives at o's offset.
        OC = 64 if li == 0 else 32   # offset of f (and c, u, m)
        OT = 64 if li == 0 else 96   # offset of o (and tau)

        # c state ping-pong tiles per direction (rows [OC:OC+H] used)
        ct = [[cpool.tile([OC + H, B], F32, tag=f"c{d}{j}",
                          name=f"ct{li}_{d}_{j}") for j in range(2)]
              for d in range(2)]
        nc.vector.memset(ct[0][1][OC:OC + H, :], 0.0)
        nc.vector.memset(ct[1][1][OC:OC + H, :], 0.0)

        def dma_in(k):
            bf = k % 2
            cols_f = slice(k * CB, (k + 1) * CB)
            nc.sync.dma_start(win[0][bf][H:K, :], xin.ap()[0:I, cols_f])
            cols_b = slice((T - (k + 1) * C) * B, (T - k * C) * B)
            nc.sync.dma_start(win[1][bf][H:K, :], xin.ap()[0:I, cols_b])

        def step_mm(d, k, tau):
            bf = k % 2
            w = win[d][bf]
            rhs = w[:, tau * B:(tau + 1) * B]
            pss = []
            for hi, (ph, gates) in enumerate(layout):
                ps = pspools[(d, hi)].tile([ph, B], F32, tag="ps",
                                           name=f"ps{li}_{d}_{hi}")
                nc.tensor.matmul(ps[:], wt[(li, d, hi)][:], rhs,
                                 start=True, stop=True)
                pss.append(ps)
            return pss

        def step_sig(d, pss):
            sgd = {}
            for hi, (ph, gates) in enumerate(layout):
                sg = spool.tile([ph, B], F32, tag=f"sg{d}{hi}",
                                name=f"sg{li}_{d}_{hi}")
                has_g = any(g == "g" for g, _ in gates)
                scale = sct[li][:] if has_g else 1.0
                nc.scalar.activation(sg[:], pss[hi][:], SIG, scale=scale,
                                     bias=bt[(li, d, hi)][:])
                for g, off in gates:
                    sgd[g] = sg[off:off + H, :]
            return sgd

        def cell_gt(d, sgd):
            # g~ = tanh(g) = 2*sigmoid(2g) - 1, rebased to partition 0
            gt = tpool.tile([H, B], F32, tag=f"g{d}", name=f"gt{li}_{d}")
            nc.vector.tensor_scalar(gt[:], sgd["g"], 2.0, 1.0, MULT, SUB)
            return gt

        def cell_m(d, sgd, cnt):
            # sf*c_old runs on gpsimd, off the DVE serial path
            cold = ct[d][(cnt + 1) % 2]
            m = tpool.tile([OC + H, B], F32, tag=f"m{d}", name=f"m{li}_{d}")
            nc.gpsimd.tensor_mul(m[OC:OC + H, :], sgd["f"],
                                 cold[OC:OC + H, :])
            return m

        def cell_u(d, sgd, gt):
            u = tpool.tile([OC + H, B], F32, tag=f"u{d}", name=f"u{li}_{d}")
            nc.vector.tensor_mul(u[OC:OC + H, :], sgd["i"], gt[:])
            return u

        def cell_add(d, m, u, cnt):
            cnew = ct[d][cnt % 2]
            nc.vector.tensor_add(cnew[OC:OC + H, :], m[OC:OC + H, :],
                                 u[OC:OC + H, :])
            return cnew

        def step_tanh(d, cnew):
            tau_t = tpool.tile([OT + H, B], F32, tag=f"t{d}",
                               name=f"tau{li}_{d}")
            nc.scalar.activation(tau_t[OT:OT + H, :], cnew[OC:OC + H, :],
                                 TANH)
            return tau_t

        def step_h(d, k, tau, sgd, tau_t):
            bf = k % 2
            w = win[d][bf]
            # h destination: shifted one block forward (fwd) / back (bwd)
            if d == 0:
                if tau < C - 1:
                    hdst = w[0:H, (tau + 1) * B:(tau + 2) * B]
                else:
                    hdst = win[0][1 - bf][0:H, 0:B]
            else:
                if tau > 0:
                    hdst = w[0:H, (tau - 1) * B:tau * B]
                else:
                    hdst = win[1][1 - bf][0:H, (C - 1) * B:C * B]
            nc.vector.tensor_mul(hdst, sgd["o"], tau_t[OT:OT + H, :])

        ro = 0  # row offset of fwd h in xo

        def writeback(k):
            bf = k % 2
            # fwd: buffer holds h_t at col t+1 -> XO rows [ro:ro+H]
            nc.sync.dma_start(
                xo.ap()[ro:ro + H, k * CB:(k * C + C - 1) * B],
                win[0][bf][0:H, B:CB])
            nc.sync.dma_start(
                xo.ap()[ro:ro + H, (k * C + C - 1) * B:(k + 1) * CB],
                win[0][1 - bf][0:H, 0:B])
            # bwd: chunk covers t in [base, base+C); h_t at col t-base-1
            base = T - (k + 1) * C
            nc.sync.dma_start(
                xo.ap()[ro + H:ro + 2 * H, (base + 1) * B:(base + C) * B],
                win[1][bf][0:H, 0:(C - 1) * B])
            nc.sync.dma_start(
                xo.ap()[ro + H:ro + 2 * H, base * B:(base + 1) * B],
                win[1][1 - bf][0:H, (C - 1) * B:CB])

        dma_in(0)
        for k in range(NCH):
            if k + 1 < NCH:
                dma_in(k + 1)
            for s in range(C):
                cnt = k * C + s
                tf, tb = s, C - 1 - s
                psf = step_mm(0, k, tf)
                psb = step_mm(1, k, tb)
                sgf = step_sig(0, psf)
                sgb = step_sig(1, psb)
                mf = cell_m(0, sgf, cnt)
                mb = cell_m(1, sgb, cnt)
                gtf = cell_gt(0, sgf)
                gtb = cell_gt(1, sgb)
                uf = cell_u(0, sgf, gtf)
                ub = cell_u(1, sgb, gtb)
                cf = cell_add(0, mf, uf, cnt)
                cb = cell_add(1, mb, ub, cnt)
                ttf = step_tanh(0, cf)
                ttb = step_tanh(1, cb)
                step_h(0, k, tf, sgf, ttf)
                step_h(1, k, tb, sgb, ttb)
            writeback(k)


def _build_fc(nc, tc, TB, xo3, fcw_t, fcb_t, out):
    """FC + 2-class softmax: p0 = sigmoid(d + bd), p1 = sigmoid(-d - bd)."""
    GRP = min(32768, TB)  # flat columns per psum group
    NG = TB // GRP
    LD = min(4096, GRP)   # flat columns per x load tile
    PSW = GRP // 64       # psum free width (2 cols per 128-chunk)
    with ExitStack() as fs:
        xp = fs.enter_context(tc.tile_pool(name="fcx", bufs=2))
        pp = fs.enter_context(tc.tile_pool(name="fcp", bufs=2,
                                           space=bass.MemorySpace.PSUM))
        op = fs.enter_context(tc.tile_pool(name="fco", bufs=2))
        for g in range(NG):
            ps = pp.tile([128, PSW], F32, tag="ps", name="fcps")
            for j in range(GRP // LD):
                xt = xp.tile([20, LD], F32, tag="xt", name="fcxt")
                base = g * GRP + j * LD
                nc.sync.dma_start(xt[:], xo3.ap()[:, base:base + LD])
                for k in range(LD // 128):
                    c = j * (LD // 128) + k  # chunk within group
                    nc.tensor.matmul(ps[:, 2 * c:2 * c + 2],
                                     xt[:, k * 128:(k + 1) * 128],
                                     fcw_t[:], start=True, stop=True)
            ot = op.tile([128, PSW], F32, tag="ot", name="fcot")
            hw = PSW // 2
            nc.scalar.activation(ot[:, 0:hw], ps[:, 0:PSW:2], SIG,
                                 bias=fcb_t[:, 0:1])
            nc.scalar.activation(ot[:, hw:PSW], ps[:, 1:PSW:2], SIG,
                                 bias=fcb_t[:, 1:2])
            for j in range(2):
                dst = out.ap()[j:j + 1, g * GRP:(g + 1) * GRP].rearrange(
                    "o (c p) -> (o p) c", p=128)
                nc.sync.dma_start(dst, ot[:, hw * j:hw * (j + 1)])


def _get_program(T, C):
    key = (T, C)
    if key not in _prog_cache:
        _prog_cache[key] = build_program(T, C)
    return _prog_cache[key]


def _prep_inputs(x, params):
    """Build the per-core input maps (numpy only)."""
    layers, fc_w, fc_b = params
    x = np.asarray(x, dtype=np.float32)
    T = x.shape[0]
    common = {}
    for li in range(4):
        I, H = INS[li], HID[li]
        for d in range(2):
            Wih, Whh, bih, bhh = [np.asarray(a, dtype=np.float32)
                                  for a in layers[li][d]]
            raw = np.concatenate([Whh.T, Wih.T], axis=0)  # (H+I, 4H)
            bias = bih + bhh
            for hi, (ph, gates) in enumerate(_gate_layout(li)):
                GIDX = {"i": 0, "f": 1, "g": 2, "o": 3}
                w = np.zeros((raw.shape[0], ph), np.float32)
                bv = np.zeros((ph, 1), np.float32)
                for g, off in gates:
                    gi = GIDX[g]
                    w[:, off:off + H] = raw[:, gi * H:(gi + 1) * H]
                    # bias rides the ACT instruction, added after the
                    # scale, so the g-gate bias is pre-doubled
                    mul = 2.0 if g == "g" else 1.0
                    bv[off:off + H, 0] = mul * bias[gi * H:(gi + 1) * H]
                common[f"w{li}_{d}_{hi}"] = w
                common[f"b{li}_{d}_{hi}"] = bv
        ph0, gates0 = _gate_layout(li)[-1]
        sc = np.ones((ph0, 1), np.float32)
        for g, off in gates0:
            if g == "g":
                sc[off:off + H] = 2.0
        common[f"sc{li}"] = sc
    fc_w = np.asarray(fc_w, dtype=np.float32)
    fc_b = np.asarray(fc_b, dtype=np.float32)
    wd = fc_w[0] - fc_w[1]
    bd = float(fc_b[0] - fc_b[1])
    common["fcw"] = np.ascontiguousarray(np.stack([wd, -wd], axis=1))
    common["fcb"] = np.stack([np.full(128, bd, np.float32),
                              np.full(128, -bd, np.float32)], axis=1)

    in_maps = []
    for c in range(NCORES):
        xs = x[:, c * B:(c + 1) * B, :]              # (T, 32, 3)
        feats = xs.transpose(2, 0, 1).reshape(3, T * B)
        m = dict(common)
        m["x0"] = np.ascontiguousarray(feats)
        in_maps.append(m)
    return in_maps


def kernel(x, params):
    x = np.asarray(x, dtype=np.float32)
    T = x.shape[0]
    C = 64 if T % 64 == 0 else 8
    nc = _get_program(T, C)
    in_maps = _prep_inputs(x, params)
    trace = bool(int(os.environ.get("KERNEL_TRACE", "0")))
    last_err = None
    for attempt in range(3):
        try:
            res = run_bass_kernel_spmd(nc, in_maps, list(range(NCORES)),
                                       trace=trace)
            break
        except Exception as e:  # transient NRT device errors recover on retry
            last_err = e
            time.sleep(2.0)
    else:
        raise last_err
    kernel.last_results = res
    outs = np.stack([np.asarray(r["out"]) for r in res.results])  # (8,2,TB)
    full = outs.reshape(NCORES, 2, T, B).transpose(2, 0, 3, 1)
    return np.ascontiguousarray(full.reshape(T * NCORES * B, 2),
                                dtype=np.float32)
